# revision 1
# baseline (speedup 1.0000x reference)
"""Trainium2 Bass kernel for nn_Attention_21122649161959 (v4, braided schedule).

RETRO-style causal self-attention block (RMSNorm -> q/kv proj -> RoPE ->
null-kv prepend -> causal masked softmax -> out proj) for
x [2, 2048, 1024], 16 heads of 64.

Sharding: 8 NeuronCores = (batch 2) x (4 head-groups of 4 heads).
Each core computes, for its batch b and heads [h0, h0+4):
    y_partial^T = Wout[h-slice]^T @ attn_out^T          [1024, 2048]
The host sums the 4 partials per batch, transposes, and adds the bias.

v4 = v3 (chunk-pipelined, bf16 E/V, restricted causal columns, diagonal
sliver trick, shifted V projection, Newton-rsqrt norm) with the engine
streams BRAIDED at instruction level: the attention j-loop for head-pair
mc yields control to a "filler" generator after each key tile, which emits
the next pair's (or next chunk's) projection matmuls between the attention
score/AV matmuls.  Since engines execute their streams in order, this is
what actually fills the PE bubbles left by the exp->mask->AV dependency
chain, instead of hoping a scheduler reorders across program order.
"""

import sys

sys.path.insert(0, "/opt/trn_rl_repo")

from contextlib import ExitStack

import numpy as np
import ml_dtypes

import concourse.bass as bass
import concourse.tile as tile
from concourse import bacc, mybir
from concourse.masks import make_identity

F32 = mybir.dt.float32
F32R = mybir.dt.float32r
BF16 = mybir.dt.bfloat16
I32 = mybir.dt.int32
AF = mybir.ActivationFunctionType
OP = mybir.AluOpType

B, N, D = 2, 2048, 1024
H, DH = 16, 64
HPC = 4
CPH = HPC * DH
NCORES = 8
NJT = 17
JPAD = NJT * 128
NCI = 4
NEG = -1e9
EPS = 1e-8
QLO = (0, 124, 252, 256)
MAGIC = 0x5F3759DF - 0x400000

TRACE = False
TRACE_KW = {}


def build_program():
    nc = bacc.Bacc(trn_type="TRN2", num_devices=NCORES)

    x_h = nc.dram_tensor("x", [N, D], F32, kind="ExternalInput")
    wq_h = nc.dram_tensor("wq", [D, CPH], F32R, kind="ExternalInput")
    wk_h = nc.dram_tensor("wk", [D, CPH], F32R, kind="ExternalInput")
    wv_h = nc.dram_tensor("wv", [D, CPH], F32R, kind="ExternalInput")
    wo_h = nc.dram_tensor("wo", [CPH, D], F32R, kind="ExternalInput")
    cos_h = nc.dram_tensor("cos2", [128, N], F32, kind="ExternalInput")
    sin_h = nc.dram_tensor("sin2", [128, N], F32, kind="ExternalInput")
    nk_h = nc.dram_tensor("nk", [2, 128], F32R, kind="ExternalInput")
    nv_h = nc.dram_tensor("nv", [1, CPH], BF16, kind="ExternalInput")
    mb_h = nc.dram_tensor("mb", [JPAD], F32, kind="ExternalInput")
    yt_h = nc.dram_tensor("yt", [D, N], F32, kind="ExternalOutput")

    with ExitStack() as ctx:
        tc = ctx.enter_context(tile.TileContext(nc))
        persist = ctx.enter_context(tc.tile_pool(name="persist", bufs=1))

        def single(shape, tag, dt=F32):
            return persist.tile(shape, dt, tag=tag, name=tag)

        qt = [single([128, N], f"qt{m}", dt=F32R) for m in range(2)]
        kt = [single([128, 2052], f"kt{m}", dt=F32R) for m in range(2)]
        vsb = [single([128, HPC, DH + 1], f"v{j}", dt=BF16) for j in range(NJT)]
        mb_sb = single([128, NJT], "mb")
        cos_sb = single([128, N], "cos")
        sin_sb = single([128, N], "sin")
        ident = single([128, 128], "ident")
        sqd = single([128, D], "sqd", dt=BF16)

        wq_sb = single([128, 8, CPH], "wqs", dt=F32R)
        wk_sb = single([128, 8, CPH], "wks", dt=F32R)
        wv_sb = single([128, 8, CPH], "wvs", dt=F32R)
        wo_sb = single([64, HPC, D], "wos", dt=F32R)

        masks = []
        for off in range(4):
            mt = single([128, 512], f"mask{off}", dt=BF16)
            nc.gpsimd.memset(mt, 1.0)
            nc.gpsimd.affine_select(
                out=mt, in_=mt, pattern=[[1, 512]],
                compare_op=OP.is_ge, fill=0.0,
                base=1 - off * 128, channel_multiplier=-1,
            )
            masks.append(mt)

        mone = single([1, 512], "m1", dt=BF16)
        nc.vector.memset(mone, 0.0)
        nc.vector.memset(mone[0:1, 511:512], 1.0)
        # dummy exp at t=0 pulls the ACT exp-table load off the critical path
        nc.scalar.activation(out=sqd[0:1, 0:1], in_=mone[0:1, 0:1], func=AF.Exp)
        e1p = [single([1, 512], f"e1p{i}", dt=BF16) for i in range(2)]
        for i in range(2):
            nc.vector.memset(e1p[i], 0.0)

        make_identity(nc, ident)

        for j in range(NJT):
            nc.vector.memset(vsb[j][:, :, DH:DH + 1], 1.0)

        with tc.tile_pool(name="xin", bufs=5) as xin, \
             tc.tile_pool(name="xnt", bufs=2) as xnt, \
             tc.tile_pool(name="stat", bufs=2) as stat, \
             tc.tile_pool(name="rope", bufs=2) as rope, \
             tc.tile_pool(name="epool", bufs=5) as epool, \
             tc.tile_pool(name="npool", bufs=3) as npool, \
             tc.tile_pool(name="upool", bufs=6) as upool, \
             tc.tile_pool(name="yout", bufs=2) as yout, \
             tc.tile_pool(name="pp", bufs=2, space="PSUM") as pp, \
             tc.tile_pool(name="pss", bufs=3, space="PSUM") as pss, \
             tc.tile_pool(name="psu", bufs=2, space="PSUM") as psu, \
             tc.tile_pool(name="ypp", bufs=1, space="PSUM") as ypp:

            xcs = {}
            utns = {}

            def norm_transpose_gen(c):
                """x loads, RMSNorm (Square on ACT, Newton-rsqrt on DVE),
                PE transposes into xc.  Yields after each transpose group."""
                xc = xnt.tile([128, 8, 512], F32R, tag="xc", name="xc")
                xcs[c] = xc
                for tr in range(4):
                    t = 4 * c + tr
                    xt = xin.tile([128, D], F32, tag="xt", name="xt")
                    nc.sync.dma_start(out=xt, in_=x_h[t * 128:(t + 1) * 128, :])
                    ms = stat.tile([128, 1], F32, tag="ms", name="ms")
                    nc.scalar.activation(out=sqd, in_=xt, func=AF.Square,
                                         accum_out=ms)
                    mh = stat.tile([128, 1], F32, tag="mh", name="mh")
                    nc.vector.tensor_scalar(
                        out=mh, in0=ms, scalar1=0.5 / D, scalar2=0.5 * EPS * EPS,
                        op0=OP.mult, op1=OP.max,
                    )
                    r = stat.tile([128, 1], F32, tag="r", name="r")
                    nc.vector.tensor_scalar(
                        out=r, in0=mh, scalar1=0.0, scalar2=1.0,
                        op0=OP.mult, op1=OP.add,
                    )
                    for _ in range(3):
                        a = stat.tile([128, 1], F32, tag="a", name="a")
                        nc.vector.tensor_mul(out=a, in0=r, in1=r)
                        nc.vector.tensor_mul(out=a, in0=a, in1=mh)
                        nc.vector.tensor_scalar(
                            out=a, in0=a, scalar1=-1.0, scalar2=1.5,
                            op0=OP.mult, op1=OP.add,
                        )
                        nc.vector.tensor_mul(out=r, in0=r, in1=a)
                    rs = stat.tile([128, 1], F32, tag="rs", name="rs")
                    nc.vector.tensor_scalar_min(out=rs, in0=r, scalar1=1.0 / EPS)
                    nc.vector.tensor_scalar_mul(out=xt, in0=xt, scalar1=rs)
                    for g in range(2):
                        tp = pp.tile([128, 512], F32, tag="pp", name="tp")
                        for q in range(4):
                            k = 4 * g + q
                            nc.tensor.matmul(
                                tp[:, q * 128:(q + 1) * 128],
                                xt[:, k * 128:(k + 1) * 128],
                                ident,
                                is_transpose=True,
                                start=(q == 0),
                                stop=(q == 3),
                            )
                        cp = nc.scalar.copy if g == 0 else nc.vector.tensor_copy
                        cp(
                            out=xc[:, 4 * g:4 * g + 4, tr * 128:(tr + 1) * 128],
                            in_=tp.rearrange("p (a b) -> p a b", a=4),
                        )
                        yield

            def projrope_gen(c, mc):
                """q/k projection + RoPE for one head pair. Yields mid-group."""
                s0, s1 = c * 512, (c + 1) * 512
                xc = xcs[c]
                m0, m1 = mc * 128, (mc + 1) * 128
                for wsb, dst, off in ((wq_sb, qt, 0), (wk_sb, kt, 1)):
                    ps = pp.tile([128, 512], F32, tag="pp", name="ps")
                    for k in range(8):
                        nc.tensor.matmul(
                            ps, wsb[:, k, m0:m1], xc[:, k, :],
                            start=(k == 0), stop=(k == 7),
                        )
                        if k % 4 == 3:
                            yield
                    qraw = rope.tile([128, 512], F32, tag="qraw", name="qraw")
                    nc.scalar.copy(out=qraw, in_=ps)
                    shuf = rope.tile([128, 512], F32, tag="shuf", name="shuf")
                    nc.vector.stream_shuffle(
                        out=shuf, in_=qraw, mask=[i ^ 1 for i in range(32)]
                    )
                    qc = rope.tile([128, 512], F32, tag="qc", name="qc")
                    nc.vector.tensor_mul(out=qc, in0=qraw, in1=cos_sb[:, s0:s1])
                    nc.gpsimd.tensor_tensor(
                        out=shuf, in0=shuf, in1=sin_sb[:, s0:s1], op=OP.mult,
                    )
                    nc.vector.tensor_add(
                        out=dst[mc][:, off + s0:off + s1], in0=qc, in1=shuf
                    )

            def vproj_gen(c):
                """V projections (pre-shifted for tr>=1, DMA shift for tr=0,
                boundary key from this chunk's last column)."""
                xc = xcs[c]
                j0 = 4 * c
                ps = pp.tile([128, CPH], F32, tag="pp", name="psv")
                for k in range(8):
                    nc.tensor.matmul(
                        ps, xc[:, k, 0:128], wv_sb[:, k, :],
                        start=(k == 0), stop=(k == 7),
                    )
                    if k % 4 == 3:
                        yield
                vtmp = rope.tile([128, CPH], BF16, tag="vtmp", name="vtmp")
                nc.vector.tensor_copy(out=vtmp, in_=ps)
                nc.sync.dma_start(
                    out=vsb[j0][1:128, :, 0:DH],
                    in_=vtmp[0:127, :].rearrange("p (h d) -> p h d", h=HPC),
                )
                psb = pp.tile([1, CPH], F32, tag="pp", name="psb")
                for k in range(8):
                    nc.tensor.matmul(
                        psb, xc[:, k, 511:512], wv_sb[:, k, :],
                        start=(k == 0), stop=(k == 7),
                    )
                yield
                nc.vector.tensor_copy(
                    out=vsb[4 * c + 4][0:1, :, 0:DH],
                    in_=psb.rearrange("p (h d) -> p h d", h=HPC),
                )
                for tr in range(1, 4):
                    j = 4 * c + tr
                    ps = pp.tile([128, CPH], F32, tag="pp", name="psv")
                    for k in range(8):
                        nc.tensor.matmul(
                            ps,
                            xc[:, k, tr * 128 - 1:tr * 128 + 127],
                            wv_sb[:, k, :],
                            start=(k == 0), stop=(k == 7),
                        )
                        if k % 4 == 3:
                            yield
                    nc.vector.tensor_copy(
                        out=vsb[j][:, :, 0:DH],
                        in_=ps.rearrange("p (h d) -> p h d", h=HPC),
                    )

            def advance(g, n=1):
                for _ in range(n):
                    try:
                        next(g)
                    except StopIteration:
                        return

            def drain(g):
                for _ in g:
                    pass

            def attn_mc(c, mc, filler):
                """Attention for chunk c, head pair mc; braids `filler`
                pieces between key tiles."""
                s0, s1 = c * 512, (c + 1) * 512
                jl = 4 * c + 4
                uts = [
                    psu.tile([65, 512], F32, tag="ut", name=f"ut{hp}")
                    for hp in range(2)
                ]
                pend = None
                for j in range(4 * c + 4):
                    qlo = 0 if j < 4 * c else QLO[j - 4 * c]
                    sps = []
                    for hi in range(2):
                        hp = hi * 64
                        sp = pss.tile([128, 512], F32, tag="sp", name="sp")
                        nc.tensor.matmul(
                            sp[:, qlo:],
                            kt[mc][hp:hp + 64, j * 128:(j + 1) * 128],
                            qt[mc][hp:hp + 64, s0 + qlo:s1],
                            start=True, stop=True,
                        )
                        sps.append(sp)
                    if pend is not None:
                        pj, pq, pes = pend
                        for hi in range(2):
                            nc.tensor.matmul(
                                uts[hi][:, pq:],
                                vsb[pj][:, 2 * mc + hi, :],
                                pes[hi][:, pq:],
                                start=(pj == 0), stop=False,
                            )
                    es = []
                    for hi in range(2):
                        e = epool.tile([128, 512], BF16, tag="e", name="e")
                        nc.scalar.activation(
                            out=e[:, qlo:], in_=sps[hi][:, qlo:], func=AF.Exp,
                            bias=mb_sb[:, j:j + 1], scale=1.0,
                        )
                        if j >= 4 * c:
                            nc.vector.tensor_mul(
                                out=e[:, qlo:], in0=e[:, qlo:],
                                in1=masks[j - 4 * c][:, qlo:],
                            )
                        es.append(e)
                    pend = (j, qlo, es)
                    advance(filler, 1 if c == 3 else (3 if j >= 4 * c else 2))
                pj, pq, pes = pend
                for hi in range(2):
                    nc.tensor.matmul(
                        uts[hi][:, pq:],
                        vsb[pj][:, 2 * mc + hi, :],
                        pes[hi][:, pq:],
                        start=(pj == 0), stop=False,
                    )
                for hi in range(2):
                    hp = hi * 64
                    sp1 = pss.tile([1, 512], F32, tag="sp", name="sp1")
                    nc.tensor.matmul(
                        sp1,
                        kt[mc][hp:hp + 64, jl * 128:jl * 128 + 1],
                        qt[mc][hp:hp + 64, s0:s1],
                        start=True, stop=True,
                    )
                    e1 = e1p[hi]
                    nc.scalar.activation(
                        out=e1[0:1, 504:512], in_=sp1[0:1, 504:512],
                        func=AF.Exp,
                        bias=mb_sb[0:1, jl:jl + 1], scale=1.0,
                    )
                    nc.vector.tensor_mul(
                        out=e1[0:1, 504:512], in0=e1[0:1, 504:512],
                        in1=mone[0:1, 504:512],
                    )
                    nc.tensor.matmul(
                        uts[hi],
                        vsb[jl][0:1, 2 * mc + hi, :],
                        e1,
                        start=False, stop=True,
                    )
                    r1_ = npool.tile([1, 512], F32, tag="r1", name="r1")
                    nc.vector.reciprocal(out=r1_, in_=uts[hi][64:65, :])
                    rb = npool.tile([64, 512], F32, tag="rb", name="rb")
                    nc.gpsimd.partition_broadcast(rb, r1_)
                    u = upool.tile([64, 512], F32R, tag="utn", name="utn")
                    nc.vector.tensor_mul(out=u, in0=uts[hi][0:64, :], in1=rb)
                    utns[(c, 2 * mc + hi)] = u

            def outproj_gen(c):
                s0, s1 = c * 512, (c + 1) * 512
                for dc in range(8):
                    yp = ypp.tile([128, 512], F32, tag="yp", name="yp")
                    for h in range(HPC):
                        nc.tensor.matmul(
                            yp,
                            wo_sb[:, h, dc * 128:(dc + 1) * 128],
                            utns[(c, h)],
                            start=(h == 0), stop=(h == HPC - 1),
                        )
                        if h == 1:
                            yield
                    ysb = yout.tile([128, 512], F32, tag="ysb", name="ysb")
                    nc.vector.tensor_copy(out=ysb, in_=yp)
                    nc.sync.dma_start(
                        out=yt_h[dc * 128:(dc + 1) * 128, s0:s1], in_=ysb
                    )
                    yield

            def chain(*gens):
                for g in gens:
                    yield from g

            def weights_dmas():
                for m in range(2):
                    nc.sync.dma_start(
                        out=kt[m][:, 0:1],
                        in_=nk_h[m:m + 1, :].rearrange("o p -> p o")
                    )
                nc.sync.dma_start(
                    out=mb_sb, in_=mb_h.rearrange("(t p) -> p t", p=128))
                nc.sync.dma_start(
                    out=vsb[0][0:1, :, 0:DH],
                    in_=nv_h.rearrange("o (h d) -> o h d", h=HPC)
                )
                nc.sync.dma_start(
                    out=wq_sb, in_=wq_h.rearrange("(k p) c -> p k c", p=128))
                nc.sync.dma_start(
                    out=wk_sb, in_=wk_h.rearrange("(k p) c -> p k c", p=128))
                nc.sync.dma_start(
                    out=wv_sb, in_=wv_h.rearrange("(k p) c -> p k c", p=128))
                nc.sync.dma_start(out=sin_sb, in_=sin_h[:, :])
                nc.sync.dma_start(out=cos_sb, in_=cos_h[:, :])
                nc.sync.dma_start(
                    out=wo_sb, in_=wo_h.rearrange("(h p) c -> p h c", p=64))
                if False:
                    yield

            # ---- driver: chunk-0 prep eager, then braided attention ----
            prep0 = chain(norm_transpose_gen(0), weights_dmas(),
                          projrope_gen(0, 0), vproj_gen(0))
            drain(prep0)
            pending_out = None
            for c in range(NCI):
                f1 = projrope_gen(c, 1)
                if pending_out is not None and c < NCI - 1:
                    f1 = chain(f1, pending_out)
                attn_mc(c, 0, f1)
                drain(f1)
                if c < NCI - 1:
                    f2 = chain(norm_transpose_gen(c + 1),
                               projrope_gen(c + 1, 0), vproj_gen(c + 1))
                else:
                    f2 = pending_out
                attn_mc(c, 1, f2)
                drain(f2)
                pending_out = outproj_gen(c)
            drain(pending_out)

    nc.compile()
    return nc


def round_f32r(a):
    """RNE-round fp32 to the PE's FP32R format (11-bit mantissa)."""
    b = np.ascontiguousarray(a, dtype=np.float32).view(np.uint32)
    b = (b + np.uint32(0x7FF) + ((b >> np.uint32(12)) & np.uint32(1))) & np.uint32(0xFFFFF000)
    return b.view(np.float32)


def host_inputs(x, mask, freqs, g, Wq, Wkv, Wout, bout, null_kv):
    """Fold g/scale into weights and build the 8 per-core input dicts."""
    f32 = lambda a: np.ascontiguousarray(np.asarray(a, dtype=np.float32))
    x, freqs, g = f32(x), f32(freqs), f32(g)
    Wq, Wkv, Wout = f32(Wq), f32(Wkv), f32(Wout)
    null_kv = f32(null_kv)
    mask = np.asarray(mask, dtype=bool)

    scale = np.float32(DH ** -0.5)
    wq_eff = (Wq * g[:, None]) * scale
    wk_eff = Wkv[:, :H * DH] * g[:, None]
    wv_eff = Wkv[:, H * DH:] * g[:, None]

    cosT = np.ascontiguousarray(np.cos(freqs).T)
    sinT = np.sin(freqs).T.copy()
    sign = np.tile(np.array([-1.0, 1.0], np.float32), DH // 2)
    sinT *= sign[:, None]
    cos2 = np.ascontiguousarray(np.tile(cosT, (2, 1)))
    sin2 = np.ascontiguousarray(np.tile(sinT, (2, 1)))

    mbs = []
    for b in range(B):
        mb = np.full([JPAD], NEG, np.float32)
        mb[0] = 0.0
        mb[1:N + 1] = np.where(mask[b], 0.0, NEG).astype(np.float32)
        mbs.append(mb)

    nk_all = null_kv[0].reshape(H, DH)
    nv_all = null_kv[1].reshape(H, DH)

    in_maps = []
    for core in range(NCORES):
        b, hg = core // 4, core % 4
        h0 = hg * HPC
        in_maps.append({
            "x": np.ascontiguousarray(x[b]),
            "wq": round_f32r(wq_eff[:, h0 * DH:(h0 + HPC) * DH]),
            "wk": round_f32r(wk_eff[:, h0 * DH:(h0 + HPC) * DH]),
            "wv": round_f32r(wv_eff[:, h0 * DH:(h0 + HPC) * DH]),
            "wo": round_f32r(Wout[h0 * DH:(h0 + HPC) * DH, :]),
            "cos2": cos2,
            "sin2": sin2,
            "nk": round_f32r(nk_all[h0:h0 + HPC].reshape(2, 128)),
            "nv": nv_all[h0:h0 + HPC].reshape(1, CPH).astype(ml_dtypes.bfloat16),
            "mb": mbs[b],
        })
    return in_maps


_CACHE = {}


def kernel(**inputs):
    if "nc" not in _CACHE:
        _CACHE["nc"] = build_program()
    nc = _CACHE["nc"]

    in_maps = host_inputs(**inputs)

    from concourse.bass_utils import run_bass_kernel_spmd

    res = run_bass_kernel_spmd(
        nc, in_maps, core_ids=list(range(NCORES)), trace=TRACE, **TRACE_KW
    )
    _CACHE["last_result"] = res

    bout = np.asarray(inputs["bout"], dtype=np.float32)
    out = np.empty([B, N, D], np.float32)
    for b in range(B):
        acc = res.results[4 * b]["yt"].astype(np.float32)
        for c in range(4 * b + 1, 4 * b + 4):
            acc = acc + res.results[c]["yt"]
        out[b] = acc.T + bout
    return out

